# revision 1
# baseline (speedup 1.0000x reference)
"""Multi-head attention (B=2, S=2048, D=1024, H=16, causal-mask capable)
on 8 Trainium2 NeuronCores.

Sharding: batch x head-group tensor parallel. Core c handles batch b=c//4
and head group g=c%4 (4 heads, d' slice of 256). Wq/Wk/Wv are split
column-wise per head group, Wo row-wise; per-core partial outputs are
summed on host (plus bo).

Device dataflow (per core), matmul operands in bf16 (same PE rate as
fp32r -- 1 moving row/cycle -- but half the DMA/SBUF traffic), fp32 PSUM
accumulation throughout:
  - host supplies x^T (=[D, S]) per batch so contraction dims land on
    SBUF partitions with no on-device transposes
  - qT/kT [d', s] and v [s, d'] projections accumulate over D in PSUM;
    kT (replicated per head so scores run at K=128 with no PE config
    switches) is built from a compact kTa via DVE bf16 copies
  - scores^T[j, i] = kT^T-slice @ qT-slice per 128-key chunk; ACT exp
    (no max-subtraction needed: |scores| <~ 8 for unit-variance data);
    causal masking on diagonal quads: es in fp32r + gpsimd affine_select
    zero-fill (vextr keeps an fp32r copy of the diagonal v-chunks since
    the PE cannot mix 32-bit and 16-bit operands)
  - ctx^T accumulates v-chunk^T @ expS with an appended ones column so
    row 64 of PSUM carries the softmax denominator; normalize with DVE
    reciprocal + gpsimd partition_broadcast + multiply
  - output projection ctx^T-chunks @ Wo-chunks; DVE PSUM->SBUF copies
    (ACT stays free for exp); partial [S, D] to HBM

Scheduling (causal): the scores->exp->ctx chain is pipelined depth-1
(PSUM caps it), which leaves ~0.5us of ACT latency exposed per quad; the
k/v projections and the deferred output projection are therefore broken
into ~0.5-1us "filler" units and woven between scores and ctx quads so
the PE always has independent work while ACT catches up. Heads 1-3
compute their diagonal quad first (its exp+select latency hides behind
the off-diagonal quads); head 0 must run off-diagonals first since the
diagonal needs kT/v of the current tile. PSUM: scores 2x2 banks + proj 1
+ outproj 1 + ctx 2 = 8.
"""

import os
import sys

import numpy as np
import ml_dtypes

try:
    import concourse.bass as bass  # noqa: F401
except ImportError:
    sys.path.insert(0, "/opt/trn_rl_repo")

import concourse.bass as bass
import concourse.tile as tile
from concourse import bacc, mybir
from concourse.bass_utils import run_bass_kernel_spmd

# Optional NTFF profiling hook (only used when BASS_TRACE=1): the agent
# image's antenv package lacks axon_hooks, so register an equivalent.
try:
    import antenv.axon_hooks  # noqa: F401
except ImportError:
    try:
        import types

        import trn_agent_boot.trn_boot as _tb

        _h = _tb._ntff_profile_via_ctypes("/opt/axon/libaxon_pjrt.so")
        _m = types.ModuleType("antenv.axon_hooks")
        _m.get_axon_ntff_profile_hook = lambda: _h
        _m.set_axon_ntff_profile_hook = lambda h: None
        sys.modules["antenv.axon_hooks"] = _m
    except Exception:
        pass

B, S, D, H = 2, 2048, 1024, 16
DH = 64                 # head dim
HLOC = 4                # heads per core
DLOC = HLOC * DH        # 256 d' per core
KC = 8                  # contraction chunks of 128 over D
ST = 512                # s-tile (matmul moving size)
NST = S // ST           # 4
JC = S // 128           # 16 key chunks
NCORES = 8

F32 = mybir.dt.float32
F32R = mybir.dt.float32r
BF16 = mybir.dt.bfloat16
NPBF16 = ml_dtypes.bfloat16

LAST_EXEC_TIME_NS = None
_NC_CACHE = {}


def _xT_layout(x2d: np.ndarray) -> np.ndarray:
    """[S, D] -> [128, NST, KC, ST] with X[p,t,kc,s] = x[t*ST+s, kc*128+p],
    bf16. Per-partition contiguous runs of KC*ST*2 = 8KB per s-tile."""
    v = x2d.reshape(NST, ST, KC, 128).transpose(3, 0, 2, 1)
    return np.ascontiguousarray(v).astype(NPBF16)


def _w_layout(w: np.ndarray, out: bool = False) -> np.ndarray:
    """[D, DLOC] -> [128, KC, DLOC] (or [DLOC, D] -> [128, 2, D]) with
    partition = contraction index within its 128-chunk; bf16."""
    kc = 2 if out else KC
    v = w.reshape(kc, 128, w.shape[1]).transpose(1, 0, 2)
    return np.ascontiguousarray(v).astype(NPBF16)


def _build(variant: str):
    """variant: 'causal' | 'zeros' | 'general'"""
    nc = bacc.Bacc("TRN2", target_bir_lowering=False, debug=False)

    xqT = nc.declare_dram_parameter("xqT", [128, NST, KC, ST], BF16, isOutput=False)
    xkT = nc.declare_dram_parameter("xkT", [128, NST, KC, ST], BF16, isOutput=False)
    xvT = nc.declare_dram_parameter("xvT", [128, NST, KC, ST], BF16, isOutput=False)
    wq = nc.declare_dram_parameter("wq", [128, KC, DLOC], BF16, isOutput=False)
    wk = nc.declare_dram_parameter("wk", [128, KC, DLOC], BF16, isOutput=False)
    wv = nc.declare_dram_parameter("wv", [128, KC, DLOC], BF16, isOutput=False)
    wo = nc.declare_dram_parameter("wo", [128, 2, D], BF16, isOutput=False)
    bq2 = nc.declare_dram_parameter("bq2", [128, 2], F32, isOutput=False)
    bk2 = nc.declare_dram_parameter("bk2", [128, 2], F32, isOutput=False)
    bv1 = nc.declare_dram_parameter("bv1", [1, DLOC], F32, isOutput=False)
    if variant == "general":
        maskTn = nc.declare_dram_parameter("maskTn", [S, S], F32, isOutput=False)
    out_d = nc.declare_dram_parameter("out", [S, D], BF16, isOutput=True)

    Exp = mybir.ActivationFunctionType.Exp
    causal = variant == "causal"

    with tile.TileContext(nc) as tc:
        with tc.tile_pool(name="wpool", bufs=1) as wpool, \
             tc.tile_pool(name="xpool", bufs=3) as xpool, \
             tc.tile_pool(name="epool", bufs=6) as epool, \
             tc.tile_pool(name="opool", bufs=2) as opool, \
             tc.tile_pool(name="spool", bufs=1) as spool, \
             tc.tile_pool(name="mpool", bufs=1) as mpool, \
             tc.tile_pool(name="pp", bufs=2, space="PSUM") as pp, \
             tc.tile_pool(name="pjp", bufs=1, space="PSUM") as pjp, \
             tc.tile_pool(name="opp", bufs=1, space="PSUM") as opp, \
             tc.tile_pool(name="ctxp", bufs=2, space="PSUM") as ctxpool:

            # ---- phase 0: warmup + weights / biases ----
            # warmup matmuls gate only on a DVE memset (no DMA), so the
            # PE p-state ramp starts as soon as the runtime comes up and
            # runs while the big input DMAs stream in.
            warm_sb = wpool.tile([128, 32], BF16, tag="warm")
            nc.vector.memset(warm_sb[:], 1.0)
            warm_ps = pp.tile([128, 2, ST], F32, tag="mm")
            for i in range(110):
                nc.tensor.matmul(
                    warm_ps[0:32, 0, 0:32], warm_sb[:], warm_sb[:],
                    start=True, stop=True, skip_group_check=True)
            vext = wpool.tile([128, JC, HLOC, 65], BF16, tag="vext")
            nc.gpsimd.memset(vext[:, :, :, 64], 1.0)
            if causal:
                # fp32r copy of the CURRENT tile's diagonal v-chunks (the
                # diagonal ctx matmul takes fp32r es, and the PE cannot
                # mix 16/32-bit operands)
                vextr = wpool.tile([128, 4, HLOC, 65], F32R, tag="vextr")
                # fp32r memset is not in the ISA: memset f32 scratch and
                # convert-copy into the ones column
                ones_f32 = wpool.tile([128, 4 * HLOC], F32, tag="ones1")
                nc.vector.memset(ones_f32[:], 1.0)
                nc.vector.tensor_copy(
                    out=vextr[:, :, :, 64],
                    in_=ones_f32[:].rearrange("p (a b) -> p a b", b=HLOC))
            wq_sb = wpool.tile([128, KC, DLOC], BF16, tag="wq")
            wk_sb = wpool.tile([128, KC, DLOC], BF16, tag="wk")
            wv_sb = wpool.tile([128, KC, DLOC], BF16, tag="wv")
            wo_sb = wpool.tile([128, 2, D], BF16, tag="wo")
            bq_sb = wpool.tile([128, 2], F32, tag="bq")
            bk_sb = wpool.tile([128, 2], F32, tag="bk")
            bv_sb = wpool.tile([1, DLOC], F32, tag="bv")
            bvb = wpool.tile([128, DLOC], F32, tag="bvb")
            # weight-load + first x s-tile emission order matters: the DMA
            # head is bandwidth-bound, so emit a per-kc wavefront of
            # {wq, xq(0)} first -- projection matmuls start after the first
            # ~400KB instead of after the full preload.
            xq_t0 = xpool.tile([128, KC, ST], BF16, tag="xq")
            xk_t0 = xpool.tile([128, KC, ST], BF16, tag="xk")
            xv_t0 = xpool.tile([128, KC, ST], BF16, tag="xv")
            for w_d, w_sb, b_d, b_sb, x_d, x_t in (
                    (wq, wq_sb, bq2, bq_sb, xqT, xq_t0),
                    (wk, wk_sb, bk2, bk_sb, xkT, xk_t0),
                    (wv, wv_sb, bv1, bv_sb, xvT, xv_t0)):
                nc.sync.dma_start(b_sb[:], b_d[:])
                for kc2 in range(KC // 2):
                    nc.sync.dma_start(
                        w_sb[:, 2 * kc2:2 * kc2 + 2, :],
                        w_d[:, 2 * kc2:2 * kc2 + 2, :])
                    nc.sync.dma_start(
                        x_t[:, 2 * kc2:2 * kc2 + 2, :],
                        x_d[:, 0, 2 * kc2:2 * kc2 + 2, :])
            nc.gpsimd.partition_broadcast(bvb[:], bv_sb[:])
            # prefetch the t=1 x tiles right behind the t=0 wavefront
            # (no latency-critical DMAs exist to head-of-line block)

            # persistent activation tensors
            qT = wpool.tile([128, 2, S], BF16, tag="qT")
            kTa = wpool.tile([128, 2, S], BF16, tag="kTa")
            kT = wpool.tile([128, HLOC, S], BF16, tag="kT")
            ctxT = wpool.tile([128, 2, S], BF16, tag="ctxT")

            def emit_loads(t):
                if t == 0:
                    return xq_t0, xk_t0, xv_t0
                xq_t = xpool.tile([128, KC, ST], BF16, tag="xq")
                xk_t = xpool.tile([128, KC, ST], BF16, tag="xk")
                xv_t = xpool.tile([128, KC, ST], BF16, tag="xv")
                for x_d, x_t in ((xqT, xq_t), (xkT, xk_t), (xvT, xv_t)):
                    for half in range(2):
                        nc.sync.dma_start(
                            x_t[:, 4 * half:4 * half + 4, :],
                            x_d[:, t, 4 * half:4 * half + 4, :])
                return xq_t, xk_t, xv_t

            def emit_proj_q(t, xq_t):
                """Both dc chunks of q into ONE 2-bank mm tile; evacuate
                with per-partition bias adds into qT (bf16)."""
                s0 = ST * t
                ps = pp.tile([128, 2, ST], F32, tag="mm")
                for dc in range(2):
                    for kc in range(KC):
                        nc.tensor.matmul(
                            ps[:, dc, :],
                            wq_sb[:, kc, 128 * dc:128 * dc + 128],
                            xq_t[:, kc, :],
                            start=(kc == 0), stop=(kc == KC - 1))
                for dc in range(2):
                    nc.vector.tensor_scalar_add(
                        out=qT[:, dc, s0:s0 + ST], in0=ps[:, dc, :],
                        scalar1=bq_sb[:, dc:dc + 1])

            def emit_proj_k_mm(t, xk_t):
                """Both dc chunks of k through one 2-bank mm tile: 16
                uninterrupted PE matmuls that cover the boundary DVE
                backlog (normalize + evacs + qrep) without gating on the
                1-bank pjp pool (whose release waits on that same DVE
                queue)."""
                s0 = ST * t
                ps = pp.tile([128, 2, ST], F32, tag="mm", name="pkmm")
                for dc in range(2):
                    for kc in range(KC):
                        nc.tensor.matmul(
                            ps[:, dc, :],
                            wk_sb[:, kc, 128 * dc:128 * dc + 128],
                            xk_t[:, kc, :],
                            start=(kc == 0), stop=(kc == KC - 1))
                for dc in range(2):
                    nc.vector.tensor_scalar_add(
                        out=kTa[:, dc, s0:s0 + ST], in0=ps[:, dc, :],
                        scalar1=bk_sb[:, dc:dc + 1])
                    for h in (2 * dc, 2 * dc + 1):
                        src_ap = kTa[64 * (h % 2):64 * (h % 2) + 64,
                                     dc, s0:s0 + ST]
                        for rep in range(2):
                            nc.vector.tensor_copy(
                                out=kT[64 * rep:64 * rep + 64, h,
                                       s0:s0 + ST],
                                in_=src_ap)

            def make_k_units(t, xk_t):
                """k-proj as 4 filler units (dc x half of the kc range).
                The last unit of each dc evacuates into kTa and emits the
                per-head replicated kT copies (DVE, bf16 SBUF->SBUF)."""
                s0 = ST * t
                state = {}
                units = []

                def unit(dc, half):
                    def f():
                        if half == 0:
                            state[dc] = pjp.tile([128, ST], F32, tag="pj", name="pjk")
                        ps = state[dc]
                        for kc in range(4 * half, 4 * half + 4):
                            nc.tensor.matmul(
                                ps[:],
                                wk_sb[:, kc, 128 * dc:128 * dc + 128],
                                xk_t[:, kc, :],
                                start=(kc == 0), stop=(kc == KC - 1))
                        if half == 1:
                            nc.vector.tensor_scalar_add(
                                out=kTa[:, dc, s0:s0 + ST], in0=ps[:],
                                scalar1=bk_sb[:, dc:dc + 1])
                            for h in (2 * dc, 2 * dc + 1):
                                src = kTa[64 * (h % 2):64 * (h % 2) + 64,
                                          dc, s0:s0 + ST]
                                for rep in range(2):
                                    nc.vector.tensor_copy(
                                        out=kT[64 * rep:64 * rep + 64, h,
                                               s0:s0 + ST],
                                        in_=src)
                    return f

                for dc in range(2):
                    for half in range(2):
                        units.append(unit(dc, half))
                return units

            def make_v_units(t, xv_t):
                """v-proj as 8 filler units (sc x half). The last unit of
                each sc evacuates into vext (bf16) and, for causal, also
                into vextr (fp32r diagonal staging)."""
                state = {}
                units = []

                def unit(sc, half):
                    def f():
                        if half == 0:
                            state[sc] = pjp.tile([128, ST], F32, tag="pj", name="pjv")
                        ps = state[sc]
                        for kc in range(4 * half, 4 * half + 4):
                            nc.tensor.matmul(
                                ps[:, 0:DLOC],
                                xv_t[:, kc, 128 * sc:128 * sc + 128],
                                wv_sb[:, kc, :],
                                start=(kc == 0), stop=(kc == KC - 1))
                        if half == 1:
                            jc = 4 * t + sc
                            pv = ps[:, 0:DLOC].rearrange(
                                "p (h d) -> p h d", d=DH)
                            bb = bvb[:].rearrange("p (h d) -> p h d", d=DH)
                            nc.vector.tensor_tensor(
                                out=vext[:, jc, :, 0:64], in0=pv, in1=bb,
                                op=mybir.AluOpType.add)
                            if causal:
                                nc.vector.tensor_tensor(
                                    out=vextr[:, sc, :, 0:64], in0=pv,
                                    in1=bb, op=mybir.AluOpType.add)
                    return f

                for sc in range(4):
                    for half in range(2):
                        units.append(unit(sc, half))
                return units

            def make_op_units(it, final=False):
                """Deferred output projection of i-tile `it` as 8 filler
                units (sc x et): 2 matmuls + DVE copy + half-row DMA out.
                Non-final units use the 1-bank opp pool (weave spacing
                hides the serialization); the final tile uses mm-pool
                2-bank tiles so its back-to-back units overlap."""
                i0 = ST * it
                state = {}
                units = []

                def unit(sc, et):
                    def f():
                        if final:
                            if et == 0:
                                state[sc] = pp.tile([128, 2, ST], F32,
                                                    tag="mm", name="opf")
                            ps = state[sc][:, et, :]
                        else:
                            ps = opp.tile([128, ST], F32, tag="op", name="opt")[:]
                        for dc in range(2):
                            nc.tensor.matmul(
                                ps,
                                ctxT[:, dc,
                                     i0 + 128 * sc:i0 + 128 * sc + 128],
                                wo_sb[:, dc, ST * et:ST * et + ST],
                                start=(dc == 0), stop=(dc == 1))
                        if et == 0:
                            state[(sc, 'ob')] = opool.tile(
                                [128, D], BF16, tag="ob", name="ob")
                        ob = state[(sc, 'ob')]
                        if final and (sc + et) % 2 == 0:
                            nc.scalar.copy(ob[:, ST * et:ST * et + ST], ps)
                        else:
                            nc.vector.tensor_copy(
                                out=ob[:, ST * et:ST * et + ST], in_=ps)
                        nc.sync.dma_start(
                            out_d[i0 + 128 * sc:i0 + 128 * sc + 128,
                                  ST * et:ST * et + ST],
                            ob[:, ST * et:ST * et + ST])
                    return f

                for sc in range(4):
                    for et in range(2):
                        units.append(unit(sc, et))
                return units

            def make_final_dc0_units(it, store):
                """First-half (heads 0/1) partials of the last i-tile's
                outproj: ready after head 1's normalize, woven as filler
                into heads 2-3; staged to f32 SBUF."""
                i0 = ST * it
                units = []

                def unit(sc, et):
                    def f():
                        ps = pjp.tile([128, ST], F32, tag="pj",
                                      name="fdc0")[:]
                        nc.tensor.matmul(
                            ps,
                            ctxT[:, 0, i0 + 128 * sc:i0 + 128 * sc + 128],
                            wo_sb[:, 0, ST * et:ST * et + ST],
                            start=True, stop=True)
                        obf = opool.tile([128, ST], F32, tag="obf",
                                         name="obf", bufs=8)
                        store[(sc, et)] = obf
                        nc.vector.tensor_copy(out=obf[:], in_=ps)
                    return f

                for sc in range(4):
                    for et in range(2):
                        units.append(unit(sc, et))
                return units

            def emit_final_dc1(it, store):
                """Second-half (heads 2/3) matmuls + combine + out DMA;
                the only work left after the last head's normalize."""
                i0 = ST * it
                for sc in range(4):
                    mmt = pp.tile([128, 2, ST], F32, tag="mm", name="fdc1")
                    ob = opool.tile([128, D], BF16, tag="ob", name="ob")
                    for et in range(2):
                        nc.tensor.matmul(
                            mmt[:, et, :],
                            ctxT[:, 1, i0 + 128 * sc:i0 + 128 * sc + 128],
                            wo_sb[:, 1, ST * et:ST * et + ST],
                            start=True, stop=True)
                    for et in range(2):
                        nc.vector.tensor_tensor(
                            out=ob[:, ST * et:ST * et + ST],
                            in0=mmt[:, et, :], in1=store[(sc, et)][:],
                            op=mybir.AluOpType.add)
                        nc.sync.dma_start(
                            out_d[i0 + 128 * sc:i0 + 128 * sc + 128,
                                  ST * et:ST * et + ST],
                            ob[:, ST * et:ST * et + ST])

            def drain(units, n=None):
                k = len(units) if n is None else min(n, len(units))
                for _ in range(k):
                    units.pop(0)()

            def emit_qrep(h, it):
                pb = 64 * (h % 2)
                hc = h // 2
                i0 = ST * it
                qrep = epool.tile([128, ST], BF16, tag="qrep", bufs=5)
                for rep in range(2):
                    nc.vector.tensor_copy(
                        out=qrep[64 * rep:64 * rep + 64, :],
                        in_=qT[pb:pb + 64, hc, i0:i0 + ST])
                return qrep

            def emit_scores_quad(qd, h, it, qrep, diag):
                i0 = ST * it
                sps = []
                for half in range(2):
                    jc0 = 4 * qd + 2 * half
                    sp = pp.tile([128, 2, ST], F32, tag="mm")
                    for c in range(2):
                        jc = jc0 + c
                        nc.tensor.matmul(
                            sp[:, c, :],
                            kT[:, h, 128 * jc:128 * jc + 128],
                            qrep[:],
                            start=True, stop=True)
                    sps.append(sp)
                tiles = []
                for half, sp in enumerate(sps):
                    jc0 = 4 * qd + 2 * half
                    if variant == "general":
                        mt = mpool.tile([128, 2, ST], F32, tag="mask")
                        nc.sync.dma_start(
                            mt[:],
                            maskTn.rearrange("(a b) i -> b a i", b=128)
                            [:, jc0:jc0 + 2, i0:i0 + ST])
                        nc.vector.tensor_tensor(
                            out=sp[:], in0=sp[:], in1=mt[:],
                            op=mybir.AluOpType.add)
                    if diag:
                        # diagonal quad: fp32r es + gpsimd affine_select
                        es = epool.tile([128, 2, ST], F32R, tag="esd",
                                        bufs=3)
                        nc.scalar.activation(es[:], sp[:], Exp)
                        nc.gpsimd.affine_select(
                            out=es[:], in_=es[:],
                            pattern=[[-128, 2], [1, ST]],
                            compare_op=mybir.AluOpType.is_ge, fill=0.0,
                            base=i0 - 128 * jc0, channel_multiplier=-1)
                    else:
                        es = epool.tile([128, 2, ST], BF16, tag="es")
                        nc.scalar.activation(es[:], sp[:], Exp)
                    tiles.append(es)
                return tiles

            def emit_ctx_quad(qd, it, tiles, cp, h, diag, start, stop):
                """start/stop: this quad opens/closes the PSUM group."""
                for half, es in enumerate(tiles):
                    for c in range(2):
                        jc = 4 * qd + 2 * half + c
                        first = start and half == 0 and c == 0
                        last = stop and half == 1 and c == 1
                        if diag and causal:
                            v_ap = vextr[:, jc - 4 * it, h, 0:65]
                        else:
                            v_ap = vext[:, jc, h, 0:65]
                        nc.tensor.matmul(
                            cp[:], v_ap, es[:, c, :],
                            start=first, stop=last)

            def emit_normalize(h, it, cp):
                pb = 64 * (h % 2)
                hc = h // 2
                i0 = ST * it
                rs = spool.tile([1, ST], F32, tag="rs", bufs=2)
                nc.vector.tensor_copy(out=rs[:], in_=cp[64:65, :])
                rc = spool.tile([1, ST], F32, tag="rc", bufs=2)
                nc.vector.reciprocal_approx_fast(out=rc[:], in_=rs[:])
                rb = spool.tile([64, ST], F32, tag="rb", bufs=2)
                nc.gpsimd.partition_broadcast(rb[:], rc[:])
                nc.vector.tensor_tensor(
                    out=ctxT[pb:pb + 64, hc, i0:i0 + ST],
                    in0=cp[0:64, :], in1=rb[:], op=mybir.AluOpType.mult)

            def head_diag_first(h, it, qrep, filler):
                """scores order: diag, 0..it-1; ctx trails by one quad;
                filler units woven after each scores quad."""
                t = it
                cp = ctxpool.tile([65, ST], F32, tag="ctx")
                order = [t] + list(range(t))
                es_prev = None
                qd_prev = None
                for qd in order:
                    es_cur = emit_scores_quad(qd, h, it, qrep,
                                              diag=(causal and qd == t))
                    # the diagonal's exp+affine_select chain is ~1us
                    # longer than a plain exp: give it two filler units
                    drain(filler, 2 if (causal and qd == t) else 1)
                    if qd_prev is not None:
                        emit_ctx_quad(qd_prev, it, es_prev, cp, h,
                                      diag=(causal and qd_prev == t),
                                      start=(qd_prev == order[0]),
                                      stop=False)
                    es_prev, qd_prev = es_cur, qd
                emit_ctx_quad(qd_prev, it, es_prev, cp, h,
                              diag=(causal and qd_prev == t),
                              start=(qd_prev == order[0]),
                              stop=True)
                emit_normalize(h, it, cp)

            if variant == "causal":
                # ---- t = 0: everything is diagonal ----
                xq_t, xk_t, xv_t = emit_loads(0)
                emit_proj_q(0, xq_t)
                qreps = [emit_qrep(h, 0) for h in range(HLOC)]
                nxt = emit_loads(1)
                drain(make_k_units(0, xk_t))
                drain(make_v_units(0, xv_t))
                es_prev = None
                cps = {}
                for h in range(HLOC):
                    es = emit_scores_quad(0, h, 0, qreps[h], diag=True)
                    if es_prev is not None:
                        hp = h - 1
                        emit_ctx_quad(0, 0, es_prev, cps[hp], hp,
                                      diag=True, start=True, stop=True)
                        emit_normalize(hp, 0, cps[hp])
                    cps[h] = ctxpool.tile([65, ST], F32, tag="ctx", name="cph")
                    es_prev = es
                emit_ctx_quad(0, 0, es_prev, cps[HLOC - 1], HLOC - 1,
                              diag=True, start=True, stop=True)
                emit_normalize(HLOC - 1, 0, cps[HLOC - 1])
                nc.sync.dma_start(wo_sb[:], wo[:])

                for t in range(1, NST):
                    xq_t, xk_t, xv_t = nxt
                    if t + 1 < NST:
                        nxt = emit_loads(t + 1)
                    emit_proj_q(t, xq_t)
                    k_units = make_k_units(t, xk_t)
                    v_units = make_v_units(t, xv_t)
                    op_units = make_op_units(t - 1, final=False)
                    # 2 k-units right behind proj_q's matmuls: pure PE
                    # work covering the DVE pile-up at the boundary
                    # (normalize(t-1,h3) + q evacs + qrep) so the PE
                    # never idles waiting for head 0's inputs
                    drain(k_units, 2)
                    qrep0 = emit_qrep(0, t)

                    # head 0: off-diagonal quads first (k-proj woven in),
                    # then finish k and v projections, then the diagonal
                    cp0 = ctxpool.tile([65, ST], F32, tag="ctx")
                    es_prev = None
                    qd_prev = None
                    for qd in range(t):
                        es_cur = emit_scores_quad(qd, 0, t, qrep0,
                                                  diag=False)
                        drain(k_units, 2)
                        if qd_prev is not None:
                            emit_ctx_quad(qd_prev, t, es_prev, cp0, 0,
                                          diag=False,
                                          start=(qd_prev == 0), stop=False)
                        es_prev, qd_prev = es_cur, qd
                    drain(k_units)
                    # heads 1-3 qreps only now: keeps their 6 DVE copies
                    # out of the boundary window
                    qreps = [qrep0] + [emit_qrep(h, t)
                                       for h in range(1, HLOC)]
                    es_diag = emit_scores_quad(t, 0, t, qrep0, diag=True)
                    drain(v_units)
                    emit_ctx_quad(qd_prev, t, es_prev, cp0, 0, diag=False,
                                  start=(qd_prev == 0), stop=False)
                    emit_ctx_quad(t, t, es_diag, cp0, 0, diag=True,
                                  start=False, stop=True)
                    emit_normalize(0, t, cp0)

                    # heads 1-3: diagonal first; outproj(t-1) woven in.
                    # On the last tile, the final outproj's dc0 half
                    # (heads 0/1) becomes extra filler once head 1 is
                    # normalized.
                    fstore = {}
                    for h in range(1, HLOC):
                        head_diag_first(h, t, qreps[h], op_units)
                        if t == NST - 1 and h == 1:
                            op_units.extend(
                                make_final_dc0_units(t, fstore))
                    drain(op_units)

                emit_final_dc1(NST - 1, fstore)
            else:
                # full attention reads all of kT/vext: all projections
                # first, then per-tile attention with deferred outproj
                # woven between quads
                nxt = emit_loads(0)
                for t in range(NST):
                    xq_t, xk_t, xv_t = nxt
                    if t + 1 < NST:
                        nxt = emit_loads(t + 1)
                    emit_proj_q(t, xq_t)
                    drain(make_k_units(t, xk_t))
                    drain(make_v_units(t, xv_t))
                nc.sync.dma_start(wo_sb[:], wo[:])
                for t in range(NST):
                    op_units = (make_op_units(t - 1) if t > 0 else [])
                    qreps = [emit_qrep(h, t) for h in range(HLOC)]
                    for h in range(HLOC):
                        cp = ctxpool.tile([65, ST], F32, tag="ctx")
                        es_prev = None
                        qd_prev = None
                        for qd in range(4):
                            es_cur = emit_scores_quad(qd, h, t, qreps[h],
                                                      diag=False)
                            drain(op_units, 1)
                            if qd_prev is not None:
                                emit_ctx_quad(qd_prev, t, es_prev, cp, h,
                                              diag=False,
                                              start=(qd_prev == 0),
                                              stop=False)
                            es_prev, qd_prev = es_cur, qd
                        emit_ctx_quad(qd_prev, t, es_prev, cp, h,
                                      diag=False, start=(qd_prev == 0),
                                      stop=True)
                        emit_normalize(h, t, cp)
                    drain(op_units)
                drain(make_op_units(NST - 1, final=True))

    nc.compile()
    return nc


def _get_nc(variant: str):
    if variant not in _NC_CACHE:
        _NC_CACHE[variant] = _build(variant)
    return _NC_CACHE[variant]


def kernel(**inputs) -> np.ndarray:
    global LAST_EXEC_TIME_NS
    q = np.asarray(inputs["query"], np.float32)
    k = np.asarray(inputs["key"], np.float32)
    v = np.asarray(inputs["value"], np.float32)
    mask = np.asarray(inputs["mask"], np.float32).reshape(S, S)
    Wq = np.asarray(inputs["Wq"], np.float32)
    bq = np.asarray(inputs["bq"], np.float32)
    Wk = np.asarray(inputs["Wk"], np.float32)
    bk = np.asarray(inputs["bk"], np.float32)
    Wv = np.asarray(inputs["Wv"], np.float32)
    bv = np.asarray(inputs["bv"], np.float32)
    Wo = np.asarray(inputs["Wo"], np.float32)
    bo = np.asarray(inputs["bo"], np.float32)

    if not mask.any():
        variant = "zeros"
    elif np.array_equal(mask, np.triu(np.ones((S, S), np.float32), k=1)):
        variant = "causal"
    else:
        variant = "general"

    scale = np.float32(1.0 / np.sqrt(DH) / 2.0)  # /2: replicated-K scores double
    xT = {}
    for b in range(B):
        xT[("q", b)] = _xT_layout(q[b])
        xT[("k", b)] = _xT_layout(k[b])
        xT[("v", b)] = _xT_layout(v[b])
    if variant == "general":
        maskTn_np = np.ascontiguousarray(mask.T) * np.float32(-1e9)

    in_maps = []
    for c in range(NCORES):
        b, g = divmod(c, 4)
        sl = slice(g * DLOC, (g + 1) * DLOC)
        m = {
            "xqT": xT[("q", b)],
            "xkT": xT[("k", b)],
            "xvT": xT[("v", b)],
            "wq": _w_layout(Wq[:, sl] * scale),
            "wk": _w_layout(Wk[:, sl]),
            "wv": _w_layout(Wv[:, sl]),
            "wo": _w_layout(Wo[sl, :], out=True),
            "bq2": np.ascontiguousarray((bq[sl] * scale).reshape(2, 128).T),
            "bk2": np.ascontiguousarray(bk[sl].reshape(2, 128).T),
            "bv1": bv[sl].reshape(1, DLOC),
        }
        if variant == "general":
            m["maskTn"] = maskTn_np
        in_maps.append(m)

    nc = _get_nc(variant)
    trace = bool(os.environ.get("BASS_TRACE"))
    res = run_bass_kernel_spmd(nc, in_maps, core_ids=list(range(NCORES)),
                               trace=trace)
    LAST_EXEC_TIME_NS = res.exec_time_ns

    out = np.empty((B, S, D), np.float32)
    for b in range(B):
        acc = np.zeros((S, D), np.float64)
        for g in range(4):
            acc += np.asarray(res.results[4 * b + g]["out"], np.float32)
        out[b] = (acc + bo).astype(np.float32)
    return out

